# revision 14
# baseline (speedup 1.0000x reference)
"""Trainium2 Bass kernel for nn_Decoder (LSTM decoder + vocab projection).

Reference computation (B=64, S=64, E=256, H=512, V=32000):
    emb     = emb_table[target_seq]                      [B,S,E]
    lstm_in = concat([emb, ctx_broadcast], -1)           [B,S,E+H]
    pre     = lstm_in @ w_ih.T + b_ih + b_hh             [B,S,4H]
    per step: gates = pre_t + h @ w_hh.T ; LSTM update   [B,4H]
    logits  = concat([hs, ctx], -1) @ w_fc.T + b_fc      [B,S,V]

Sharding (8 cores): pure batch-parallel, 8 batches/core, NO collectives.
Each core computes the FULL vocab projection for its own 512 local
tokens (n = t*8 + b, t-major).

Structure (v4):
  - `pre` and the ctx half of the FC (ctxl[v,b] = ctx_b @ w_fc[v,H:] +
    b_fc) are computed on the HOST in f32; `pre` ships as bf16, and
    ctxl is ADDED ON THE HOST during assembly, so the device's vocab
    projection is just w1 @ hs.  Device DMA per core: w1 (fp8,
    16.4MB), pre (2.1MB), whh (1MB) in; logits out.
  - recurrence: h kept in fp8 (feeds both DoubleRow h@w_hh and the FC
    rhs), c in f32, gate blocks host-permuted to [i, f, o, g].  The
    64-step chain (PE -> ACT -> DVE -> ACT -> DVE) is the critical
    path; nothing else is scheduled on ACT/DVE while it runs.
  - FC unit = (vtile-quad, 128-token quarter): 8 fp8 DoubleRow matmuls
    into a [128,4,128] f32 psum bank.  Two kinds of vtile-quads:
      f32-quads:  psum is DMA'd STRAIGHT to DRAM as f32 (no engine
                  evacuation at all) - these overlap the recurrence,
                  token-quarter q starting after step 16(q+1);
      bf16-quads: ACT/DVE copy psum -> bf16 fo (two quarters per fo),
                  stored bf16 - these run in the tail when ACT/DVE
                  are free.
  - stores round-robin over the SP and Pool DMA queues; host upcasts /
    merges the two logit tensors and adds ctxl.
"""

import numpy as np
import ml_dtypes

VOCAB, EMBED, HIDDEN = 32000, 256, 512
B, S = 64, 64
NCORES = 8
BL = B // NCORES          # 8 local batches
TOKL = S * BL             # 512 local tokens
G4 = 4 * HIDDEN           # 2048
KH = HIDDEN // 128        # 4 k-tiles
GT = G4 // 128            # 16 gate tiles
VT = VOCAB // 128         # 250 vocab tiles
NQ = VT // 4              # 62 full vtile-quads (quad 62 is a pair)
QTOK = TOKL // 4          # 128 tokens per FC quarter
WG = 10                   # vtiles per w1 load group
NWG = VT // WG            # 25 groups


BF16 = ml_dtypes.bfloat16
FP8 = ml_dtypes.float8_e4m3

_CACHE = {}


def _build_program():
    import concourse.bass as bass
    import concourse.mybir as mybir
    import concourse.tile as tile
    from concourse import bacc

    bf = mybir.dt.bfloat16
    f8 = mybir.dt.float8e4
    f32 = mybir.dt.float32
    AF = mybir.ActivationFunctionType
    DR = mybir.MatmulPerfMode.DoubleRow

    nc = bacc.Bacc(
        "TRN2",
        target_bir_lowering=False,
        debug=False,
        num_devices=NCORES,
    )

    # ---- DRAM I/O ----------------------------------------------------
    pre_d = nc.dram_tensor("pre_d", [GT, 128, TOKL], bf, kind="ExternalInput").ap()
    whh_d = nc.dram_tensor("whh_d", [KH, 128, G4], f8, kind="ExternalInput").ap()
    h0_d = nc.dram_tensor("h0_d", [128, KH, BL], f8, kind="ExternalInput").ap()
    c0_d = nc.dram_tensor("c0_d", [128, KH, BL], f32, kind="ExternalInput").ap()
    id_d = nc.dram_tensor("id_d", [128, 128], f8, kind="ExternalInput").ap()
    w1_d = nc.dram_tensor("w1_d", [KH, 128, VOCAB], f8, kind="ExternalInput").ap()
    log_d = nc.dram_tensor("log_d", [VT, 128, TOKL], bf, kind="ExternalOutput").ap()

    with tile.TileContext(nc) as tc, \
         tc.tile_pool(name="singles", bufs=1) as sg:
        # ---- persistent SBUF tensors ---------------------------------
        pre_sb = sg.tile([128, GT, TOKL], bf, name="pre_sb", tag="pre_sb")
        whh_sb = sg.tile([128, KH, G4], f8, name="whh_sb", tag="whh_sb")
        h0_sb = sg.tile([128, KH, BL], f8, name="h0_sb", tag="h0_sb")
        c0_sb = sg.tile([128, KH, BL], f32, name="c0_sb", tag="c0_sb")
        id_sb = sg.tile([128, 128], f8, name="id_sb", tag="id_sb")
        hs_sb = sg.tile([128, KH, TOKL], f8, name="hs_sb", tag="hs_sb")
        w1_sb = sg.tile([128, KH, VOCAB], f8, name="w1_sb", tag="w1_sb")

        # ---- input DMAs ----------------------------------------------
        nc.gpsimd.dma_start(out=whh_sb[:], in_=whh_d.rearrange("k p n -> p k n"))
        nc.sync.dma_start(out=id_sb[:], in_=id_d)
        nc.sync.dma_start(out=h0_sb[:], in_=h0_d)
        nc.sync.dma_start(out=c0_sb[:], in_=c0_d)
        for pc in range(4):
            ts = pc * (TOKL // 4)
            nc.sync.dma_start(
                out=pre_sb[:, :, ts:ts + TOKL // 4],
                in_=pre_d[:, :, ts:ts + TOKL // 4].rearrange("g p n -> p g n"),
            )

        def emit_w1load(g):
            vs = g * WG * 128
            eng = nc.sync if g % 2 == 0 else nc.gpsimd
            eng.dma_start(
                out=w1_sb[:, :, vs:vs + WG * 128],
                in_=w1_d[:, :, vs:vs + WG * 128].rearrange("k p n -> p k n"),
            )

        with (
            tc.tile_pool(name="act", bufs=3) as actp,
            tc.tile_pool(name="cst", bufs=2) as cstp,
            tc.tile_pool(name="tmp", bufs=3) as tmpp,
            tc.tile_pool(name="fout", bufs=4) as foutp,
            tc.tile_pool(name="pgate", bufs=2, space="PSUM") as pgate,
            tc.tile_pool(name="pfc", bufs=5, space="PSUM") as pfc,
        ):
            # ---- FC matmuls for one (quad, quarter) ------------------
            def fc_matmuls(q, qt):
                v0 = 4 * q
                nv = 4 if q < NQ else 2
                ts = qt * QTOK
                ps = pfc.tile([128, 4, QTOK], f32, tag="pfc")
                for j in range(nv):
                    for kq in range(2):
                        nc.tensor.matmul(
                            ps[:, j],
                            lhsT=w1_sb[:, 2 * kq:2 * kq + 2,
                                       (v0 + j) * 128:(v0 + j + 1) * 128],
                            rhs=hs_sb[:, 2 * kq:2 * kq + 2, ts:ts + QTOK],
                            perf_mode=DR,
                            start=(kq == 0),
                            stop=(kq == 1),
                        )
                return ps, nv

            # bf16 evacuation of one (quad, quarter): ACT or DVE
            # copies psum -> fo; the second quarter of each half
            # triggers the bf16 store (512B-contiguous DRAM runs).
            fo_cur = {}

            def emit_unit(q, qt, eng, st, nvp=2):
                ps, nv = fc_matmuls(q, qt)
                v0 = 4 * q
                hh = qt // 2
                key = (q, hh)
                if key not in fo_cur:
                    fo_cur[key] = foutp.tile([128, 4, 2 * QTOK], bf,
                                             name="fo", tag="fo")
                fo = fo_cur[key]
                for vp in range(0, nv, nvp):
                    dst = fo[:, vp:vp + nvp,
                             (qt % 2) * QTOK:(qt % 2 + 1) * QTOK]
                    e = eng if vp == 0 else 1 - eng
                    if e == 0:
                        nc.scalar.copy(dst, ps[:, vp:vp + nvp])
                    else:
                        nc.vector.tensor_scalar_add(dst, ps[:, vp:vp + nvp],
                                                    0.0)
                if qt % 2 == 1:
                    del fo_cur[key]
                    st.dma_start(
                        out=log_d[v0:v0 + nv, :,
                                  hh * 2 * QTOK:(hh + 1) * 2 * QTOK]
                        .rearrange("v p n -> p v n"),
                        in_=fo[:, 0:nv],
                    )

            # ---- unit schedules --------------------------------------
            # During the recurrence: a strictly limited trickle of units
            # (their copies fit the chain's idle gaps).  fo tiles close
            # only on the odd quarter, so pair every open (qt=0) with a
            # close (qt=1) to bound live fo tiles.  Rest in tail.
            rec_units = [(0, 0), (1, 0), (2, 0)]
            for q in range(3, NQ + 1):
                rec_units += [(q - 3, 1), (q, 0)]
            fidx = [0]

            def emit_rec_units(n, tmax):
                for _ in range(n):
                    u = fidx[0]
                    if u >= len(rec_units):
                        return
                    q, qt = rec_units[u]
                    if 16 * (qt + 1) > tmax:
                        return
                    fidx[0] += 1
                    emit_unit(q, qt, u % 2, nc.sync if u % 2 else nc.gpsimd)

            filler = [[] for _ in range(S + 1)]
            for t in range(18, S):
                filler[t].append(lambda t=t: emit_rec_units(1, t))

            # w1 loads all emitted up front: the two queues drain them
            # behind the small recurrence-critical loads.
            for g in range(NWG):
                emit_w1load(g)

            # ---- the recurrence --------------------------------------
            c_prev = c0_sb
            for t in range(S):
                gp = pgate.tile([128, GT, BL], f32, tag="gates")
                # pre contribution via identity matmul (accumulate base)
                nc.tensor.matmul(
                    gp[:],
                    lhsT=id_sb[:],
                    rhs=pre_sb[:, :, t * BL:(t + 1) * BL],
                    start=True,
                    stop=False,
                )
                rhs_src = h0_sb if t == 0 else hs_sb
                roff = 0 if t == 0 else (t - 1) * BL
                for gt in range(GT):
                    for kq in range(2):
                        nc.tensor.matmul(
                            gp[:, gt],
                            lhsT=whh_sb[:, 2 * kq:2 * kq + 2,
                                        gt * 128:(gt + 1) * 128],
                            rhs=rhs_src[:, 2 * kq:2 * kq + 2,
                                        roff:roff + BL],
                            perf_mode=DR,
                            start=False,
                            stop=(gt == GT - 1 and kq == 1),
                        )
                # activations: one sigmoid over ALL 16 gate blocks;
                # tanh(g) = 2*sigmoid(2g) - 1, the 2x fold being done
                # on the host (g rows of whh / pre pre-scaled by 2).
                s16 = actp.tile([128, GT, BL], bf, tag="s16")
                nc.scalar.activation(s16[:], gp[:], AF.Sigmoid)

                tg = tmpp.tile([128, KH, BL], bf, tag="tg")
                t1 = tmpp.tile([128, KH, BL], f32, tag="t1")
                t2 = tmpp.tile([128, KH, BL], f32, tag="t2")
                c_new = cstp.tile([128, KH, BL], f32, tag="c")
                tcn = actp.tile([128, KH, BL], bf, tag="tc")
                nc.vector.tensor_scalar(tg[:], s16[:, 3 * KH:GT], 2.0, -1.0,
                                        mybir.AluOpType.mult,
                                        mybir.AluOpType.add)
                nc.vector.tensor_mul(t1[:], s16[:, 0:KH], tg[:])
                nc.vector.tensor_mul(t2[:], s16[:, KH:2 * KH], c_prev[:])
                nc.vector.tensor_add(c_new[:], t1[:], t2[:])
                nc.scalar.activation(tcn[:], c_new[:], AF.Tanh)
                # h (fp8) in two k-pair halves so step t+1's first DR
                # matmul can start before the second half lands
                hslice = hs_sb[:, :, t * BL:(t + 1) * BL]
                nc.vector.tensor_mul(
                    hslice[:, 0:2], s16[:, 2 * KH:2 * KH + 2], tcn[:, 0:2])
                nc.vector.tensor_mul(
                    hslice[:, 2:4], s16[:, 2 * KH + 2:3 * KH], tcn[:, 2:4])
                c_prev = c_new

                for th in filler[t]:
                    th()

            # ---- tail ------------------------------------------------
            done = set(rec_units[:fidx[0]])
            tail = [(q, qt) for q in range(NQ + 1) for qt in range(4)
                    if (q, qt) not in done]
            nst = [0]
            for u, (q, qt) in enumerate(tail):
                st = nc.sync if nst[0] % 2 else nc.gpsimd
                nst[0] += 1
                emit_unit(q, qt, u % 2, st)

    nc.compile()
    return nc


def _get_nc():
    if "nc" not in _CACHE:
        _CACHE["nc"] = _build_program()
    return _CACHE["nc"]


def _block128(a):
    """[K, N] -> [K//128, 128, N] contiguous blocks."""
    k, n = a.shape
    return np.ascontiguousarray(a.reshape(k // 128, 128, n))


def _t_layout(a, dt):
    """[BL, 512] state -> [128, KH, BL] transposed tile layout."""
    return np.ascontiguousarray(a.T.reshape(KH, 128, BL).transpose(1, 0, 2)
                                ).astype(dt)


def _prep_in_maps(target_seq, context, h, c, emb_table, w_ih, w_hh, b_ih,
                  b_hh, w_fc, b_fc):
    target_seq = np.asarray(target_seq)
    context = np.asarray(context, dtype=np.float32)
    h = np.asarray(h, dtype=np.float32)
    c = np.asarray(c, dtype=np.float32)
    emb_table = np.asarray(emb_table, dtype=np.float32)
    w_ih = np.asarray(w_ih, dtype=np.float32)
    w_hh = np.asarray(w_hh, dtype=np.float32)
    b_ih = np.asarray(b_ih, dtype=np.float32)
    b_hh = np.asarray(b_hh, dtype=np.float32)
    w_fc = np.asarray(w_fc, dtype=np.float32)
    b_fc = np.asarray(b_fc, dtype=np.float32)

    # gate-block permutation [i, f, g, o] -> [i, f, o, g]
    perm = np.concatenate([
        np.arange(0, HIDDEN),                    # i
        np.arange(HIDDEN, 2 * HIDDEN),           # f
        np.arange(3 * HIDDEN, 4 * HIDDEN),       # o
        np.arange(2 * HIDDEN, 3 * HIDDEN),       # g
    ])
    w_hh_p = w_hh[perm].copy()
    bias_p = (b_ih + b_hh)[perm]
    # scale the g-gate rows by 2: tanh(g) = 2*sigmoid(2g) - 1
    w_hh_p[3 * HIDDEN:] *= 2.0

    whh_d = _block128(w_hh_p.T.astype(FP8))            # [4,128,2048]
    id_d = np.eye(128, dtype=FP8)
    w1_d = _block128(np.ascontiguousarray(w_fc[:, :HIDDEN].T).astype(FP8))

    # host: pre = lstm_in @ w_ih_p.T + bias for ALL tokens  [B,S,4H]
    emb = emb_table[target_seq]                        # [B,S,E] f32
    lstm_in = np.concatenate(
        [emb, np.broadcast_to(context[:, None, :], (B, S, HIDDEN))], axis=2
    ).reshape(B * S, EMBED + HIDDEN)                   # [B*S, 768]
    pre_all = lstm_in @ w_ih[perm].T + bias_p          # [B*S, 2048]
    pre_all[:, 3 * HIDDEN:] *= 2.0
    pre_all = pre_all.reshape(B, S, G4)

    in_maps = []
    for cid in range(NCORES):
        bs = slice(cid * BL, (cid + 1) * BL)
        # local tokens n = t*8 + b (t-major)
        pre_loc = (pre_all[bs].transpose(1, 0, 2)      # [S, BL, 2048]
                   .reshape(TOKL, G4))
        pre_d = _block128(np.ascontiguousarray(pre_loc.T).astype(BF16))
        in_maps.append({
            "pre_d": pre_d,
            "whh_d": whh_d,
            "h0_d": _t_layout(h[bs], FP8),
            "c0_d": _t_layout(c[bs], np.float32),
            "id_d": id_d,
            "w1_d": w1_d,
        })
    return in_maps


def _ctx_logits(context, w_fc, b_fc):
    """Host ctx half of the FC: [B, V] f32."""
    context = np.asarray(context, dtype=np.float32)
    w_fc = np.asarray(w_fc, dtype=np.float32)
    b_fc = np.asarray(b_fc, dtype=np.float32)
    return context @ w_fc[:, HIDDEN:].T + b_fc


def _assemble(results, ctxl_all):
    """Merge bf16/f32 device logits, add host ctx term -> [B, S, V]."""
    logits = np.empty((B, S, VOCAB), dtype=np.float32)
    for cid in range(NCORES):
        out_c = (results[cid]["log_d"].astype(np.float32)
                 .reshape(VOCAB, S, BL))               # [V, t, b]
        ctx_c = ctxl_all[cid * BL:(cid + 1) * BL]      # [BL, V]
        logits[cid * BL:(cid + 1) * BL] = (
            out_c.transpose(2, 1, 0) + ctx_c[:, None, :])
    return np.ascontiguousarray(logits)


def kernel(target_seq, context, h, c, emb_table, w_ih, w_hh, b_ih, b_hh,
           w_fc, b_fc):
    from concourse.bass_utils import run_bass_kernel_spmd

    in_maps = _prep_in_maps(target_seq, context, h, c, emb_table, w_ih,
                            w_hh, b_ih, b_hh, w_fc, b_fc)
    ctxl_all = _ctx_logits(context, w_fc, b_fc)
    nc = _get_nc()
    res = run_bass_kernel_spmd(nc, in_maps, core_ids=list(range(NCORES)))
    return _assemble(res.results, ctxl_all)
